# revision 2
# baseline (speedup 1.0000x reference)
"""CIF (continuous integrate-and-fire) segment-reduce kernel for Trainium2.

Strategy
--------
The CIF recurrence over T is sequential only in the *scalar* alpha stream
(B*T = 64K f32 values).  The heavy part - accumulating alpha-weighted hidden
vectors into label slots - is a banded matmul  out[b] = W_b @ hidden[b]
with W_b in R^{L x T} holding at most 2 nonzeros per column:

  * timestep t contributes weight cur_t to the slot of the next fire at-or-
    after t  (slotA), and
  * weight rem_t to the slot of the next fire strictly after t (slotB,
    nonzero only at fire steps).

The host replicates the reference's f32 scan bit-exactly (same IEEE ops in
the same order) to derive (slotA, slotB, wA, wB) per timestep; the device
expands these into 128x256 weight tiles with two fused tensor_scalar ops
(iota == slot) * w and contracts against hidden with fp32 matmuls
accumulated in PSUM.

Sharding: pure data parallelism - batch 32 is split 4-per-core across the
8 NeuronCores; no communication.
"""

import sys

if "/opt/trn_rl_repo" not in sys.path:
    sys.path.insert(0, "/opt/trn_rl_repo")

import numpy as np

import concourse.bass as bass
import concourse.tile as tile
from concourse import bacc, mybir
from concourse.bass_utils import run_bass_kernel_spmd

# Problem constants (hardcoded per the task contract).
B, T, H, L = 32, 2048, 512, 256
N_CORES = 8
B_PER_CORE = B // N_CORES          # 4
TCHUNK = 128                       # timesteps per matmul contraction chunk
NCHUNK = T // TCHUNK               # 16
F32 = mybir.dt.float32

_compiled = None  # cached (nc, out_name)


def host_scan(alphas: np.ndarray) -> tuple[np.ndarray, ...]:
    """Replicate the reference's sequential f32 scan exactly.

    Returns slotA, slotB (f32 label indices) and wA, wB (f32 weights),
    each [B, T]:  out[b, l] = sum_t (slotA==l)*wA*h_t + (slotB==l)*wB*h_t.
    """
    Bn, Tn = alphas.shape
    one = np.float32(1.0)
    thr = np.float32(0.95)
    integrate = np.zeros(Bn, np.float32)
    fire_all = np.zeros((Bn, Tn), bool)
    cur_all = np.empty((Bn, Tn), np.float32)
    rem_all = np.empty((Bn, Tn), np.float32)
    for t in range(Tn):
        at = alphas[:, t]
        dist = one - integrate
        integrate = integrate + at
        fire = integrate > thr
        integrate = np.where(fire, integrate - one, integrate)
        cur = np.where(fire, dist, at)
        fire_all[:, t] = fire
        cur_all[:, t] = cur
        rem_all[:, t] = at - cur

    k_t = np.cumsum(fire_all, axis=1)        # fires up to and including t
    n_before = k_t - fire_all                # fires strictly before t
    total = k_t[:, -1:]
    slotA = np.minimum(n_before, L - 1).astype(np.float32)
    slotB = np.minimum(k_t, L - 1).astype(np.float32)
    wA = np.where(n_before < total, cur_all, np.float32(0.0))
    wB = np.where(k_t < total, rem_all, np.float32(0.0))
    return slotA, slotB, wA, wB


def build_program():
    """Static (input-independent) per-core Bass program."""
    nc = bacc.Bacc("TRN2", target_bir_lowering=False, debug=False)

    hid_d = nc.dram_tensor("hidden", [B_PER_CORE, T, H], F32, kind="ExternalInput")
    aux_d = nc.dram_tensor("aux", [B_PER_CORE, 128, 64], F32, kind="ExternalInput")
    iota_d = nc.dram_tensor("iota", [128, L], F32, kind="ExternalInput")
    out_d = nc.dram_tensor("out", [B_PER_CORE, L, H], F32, kind="ExternalOutput")

    eq = mybir.AluOpType.is_equal
    mul = mybir.AluOpType.mult

    with tile.TileContext(nc) as tc:
        with (
            tc.tile_pool(name="const", bufs=1) as constp,
            tc.tile_pool(name="auxp", bufs=2) as auxp,
            tc.tile_pool(name="hid", bufs=4) as hidp,
            tc.tile_pool(name="band", bufs=3) as bandp,
            tc.tile_pool(name="outp", bufs=2) as outp,
            tc.tile_pool(name="psum", bufs=2, space="PSUM") as psump,
        ):
            iota_t = constp.tile([128, L], F32)
            nc.sync.dma_start(iota_t[:], iota_d[:])

            for i in range(B_PER_CORE):
                auxt = auxp.tile([128, 64], F32)
                nc.sync.dma_start(auxt[:], aux_d[i])
                ps0 = psump.tile([128, H], F32)
                ps1 = psump.tile([128, H], F32)
                for c in range(NCHUNK):
                    ht = hidp.tile([128, H], F32)
                    nc.sync.dma_start(ht[:], hid_d[i, c * TCHUNK : (c + 1) * TCHUNK, :])
                    # band tile [128 t, 256 l]: (iota==slotA)*wA + (iota==slotB)*wB
                    t1 = bandp.tile([128, L], F32, tag="t1")
                    nc.vector.tensor_scalar(
                        t1[:], iota_t[:], auxt[:, c : c + 1],
                        auxt[:, 32 + c : 33 + c], eq, mul,
                    )
                    t2 = bandp.tile([128, L], F32, tag="t2")
                    nc.vector.tensor_scalar(
                        t2[:], iota_t[:], auxt[:, 16 + c : 17 + c],
                        auxt[:, 48 + c : 49 + c], eq, mul,
                    )
                    wt = bandp.tile([128, L], F32, tag="wt")
                    nc.gpsimd.tensor_add(wt[:], t1[:], t2[:])
                    nc.tensor.matmul(
                        ps0[:], wt[:, 0:128], ht[:],
                        start=(c == 0), stop=(c == NCHUNK - 1),
                    )
                    nc.tensor.matmul(
                        ps1[:], wt[:, 128:256], ht[:],
                        start=(c == 0), stop=(c == NCHUNK - 1),
                    )
                o0 = outp.tile([128, H], F32, tag="o0")
                nc.scalar.copy(o0[:], ps0[:])
                o1 = outp.tile([128, H], F32, tag="o1")
                nc.scalar.copy(o1[:], ps1[:])
                nc.sync.dma_start(out_d[i, 0:128, :], o0[:])
                nc.sync.dma_start(out_d[i, 128:256, :], o1[:])

    nc.compile()
    return nc, out_d.name


def make_in_maps(hidden: np.ndarray, alphas: np.ndarray) -> list[dict]:
    slotA, slotB, wA, wB = host_scan(alphas)
    # aux[b] = [128, 64]: per 128-row chunk c, col k*16+c holds arr_k[b, c*128+p]
    aux = np.concatenate(
        [a.reshape(B, NCHUNK, 128).transpose(0, 2, 1) for a in (slotA, slotB, wA, wB)],
        axis=2,
    ).astype(np.float32)  # [B, 128, 64]
    iota = np.ascontiguousarray(
        np.broadcast_to(np.arange(L, dtype=np.float32), (128, L))
    )
    in_maps = []
    for j in range(N_CORES):
        sl = slice(j * B_PER_CORE, (j + 1) * B_PER_CORE)
        in_maps.append(
            {
                "hidden": np.ascontiguousarray(hidden[sl]),
                "aux": np.ascontiguousarray(aux[sl]),
                "iota": iota,
            }
        )
    return in_maps


def _get_compiled():
    global _compiled
    if _compiled is None:
        _compiled = build_program()
    return _compiled


def run_sharded(hidden: np.ndarray, alphas: np.ndarray, trace: bool = False, **kw):
    """Run the SPMD kernel; returns (out [B,L,H] f32, BassKernelResults)."""
    nc, out_name = _get_compiled()
    in_maps = make_in_maps(hidden, alphas)
    res = run_bass_kernel_spmd(nc, in_maps, list(range(N_CORES)), trace=trace, **kw)
    out = np.concatenate([r[out_name] for r in res.results], axis=0)
    return out, res


def kernel(hidden, alphas, num_labels=L) -> np.ndarray:
    hidden = np.asarray(hidden, dtype=np.float32)
    alphas = np.asarray(alphas, dtype=np.float32)
    assert hidden.shape == (B, T, H) and alphas.shape == (B, T)
    assert int(num_labels) == L
    out, _ = run_sharded(hidden, alphas)
    return out


# revision 6
# speedup vs baseline: 1.5141x; 1.5141x over previous
"""CIF (continuous integrate-and-fire) segment-reduce kernel for Trainium2.

Strategy
--------
The CIF recurrence over T is sequential only in the *scalar* alpha stream
(B*T = 64K f32 values).  The heavy part - accumulating alpha-weighted hidden
vectors into label slots - is a banded matmul  out[b] = W_b @ hidden[b]
with W_b in R^{L x T} holding at most 2 nonzeros per column:

  * timestep t contributes weight cur_t to the slot of the next fire at-or-
    after t (slotA), and
  * weight rem_t to the slot of the next fire strictly after t (slotB =
    slotA+1, nonzero only at fire steps).

The host replicates the reference's f32 scan bit-exactly (same IEEE ops in
the same order) to derive (slotA, slotB, wA, wB) per timestep; the device
expands these into weight tiles with fused tensor_scalar (iota==slot)*w ops
and contracts against hidden with fp32 matmuls accumulated in PSUM.

Because alphas rows sum to exactly L, slot(t) tracks 0.125*t with only a
few slots of drift, so each 128-timestep chunk touches a narrow slot band.
The fast path exploits this: chunks are grouped 4-at-a-time and their
contributions land in a static 128-slot PSUM window per group (bases
0/32/96/128), one matmul per chunk; overlapping windows are then combined
into the [256, H] output with a handful of PSUM->SBUF copies/adds.  The
host verifies every contribution fits its window and falls back to a
generic full-width (two matmuls per chunk) program otherwise.

Sharding: pure data parallelism - batch 32 is split 4-per-core across the
8 NeuronCores; no communication.
"""

import sys

if "/opt/trn_rl_repo" not in sys.path:
    sys.path.insert(0, "/opt/trn_rl_repo")

import numpy as np

import concourse.bass as bass
import concourse.tile as tile
from concourse import bacc, mybir
from concourse.bass_utils import run_bass_kernel_spmd

# Problem constants (hardcoded per the task contract).
B, T, H, L = 32, 2048, 512, 256
N_CORES = 8
B_PER_CORE = B // N_CORES          # 4
TCHUNK = 128                       # timesteps per matmul contraction chunk
NCHUNK = T // TCHUNK               # 16
GROUP = 4                          # chunks per PSUM window group
NGROUP = NCHUNK // GROUP           # 4
WIN_BASE = (0, 32, 96, 128)        # static slot-window base per group
F32 = mybir.dt.float32

_compiled = {}  # variant -> (nc, out_name)


def host_scan(alphas: np.ndarray) -> tuple[np.ndarray, ...]:
    """Replicate the reference's sequential f32 scan exactly.

    Returns slotA, slotB (f32 label indices) and wA, wB (f32 weights),
    each [B, T]:  out[b, l] = sum_t (slotA==l)*wA*h_t + (slotB==l)*wB*h_t.
    """
    Bn, Tn = alphas.shape
    one = np.float32(1.0)
    thr = np.float32(0.95)
    integrate = np.zeros(Bn, np.float32)
    fire_all = np.zeros((Bn, Tn), bool)
    cur_all = np.empty((Bn, Tn), np.float32)
    rem_all = np.empty((Bn, Tn), np.float32)
    for t in range(Tn):
        at = alphas[:, t]
        dist = one - integrate
        integrate = integrate + at
        fire = integrate > thr
        integrate = np.where(fire, integrate - one, integrate)
        cur = np.where(fire, dist, at)
        fire_all[:, t] = fire
        cur_all[:, t] = cur
        rem_all[:, t] = at - cur

    k_t = np.cumsum(fire_all, axis=1)        # fires up to and including t
    n_before = k_t - fire_all                # fires strictly before t
    total = k_t[:, -1:]
    slotA = np.minimum(n_before, L - 1).astype(np.float32)
    slotB = np.minimum(k_t, L - 1).astype(np.float32)
    wA = np.where(n_before < total, cur_all, np.float32(0.0))
    wB = np.where(k_t < total, rem_all, np.float32(0.0))
    return slotA, slotB, wA, wB


def _pack_aux(slotA, slotB, wA, wB):
    """[B, 128, 64]: col k*16+c of row p holds arr_k[b, c*128+p]."""
    return np.ascontiguousarray(
        np.concatenate(
            [a.reshape(B, NCHUNK, 128).transpose(0, 2, 1)
             for a in (slotA, slotB, wA, wB)],
            axis=2,
        ).astype(np.float32)
    )


def _window_ok(slotA, slotB, wA, wB) -> bool:
    """Every nonzero contribution must land inside its chunk-group window."""
    for g in range(NGROUP):
        base = WIN_BASE[g]
        sl = slice(g * GROUP * TCHUNK, (g + 1) * GROUP * TCHUNK)
        for s, w in ((slotA[:, sl], wA[:, sl]), (slotB[:, sl], wB[:, sl])):
            m = w != 0
            if m.any():
                v = s[m]
                if v.min() < base or v.max() > base + 127:
                    return False
    return True


def build_program_windowed():
    """Fast path: one matmul per chunk into a static 128-slot PSUM window."""
    nc = bacc.Bacc("TRN2", target_bir_lowering=False, debug=False)

    hid_d = nc.dram_tensor("hidden", [B_PER_CORE, T, H], F32, kind="ExternalInput")
    aux_d = nc.dram_tensor("aux", [B_PER_CORE, 128, 64], F32, kind="ExternalInput")
    iota_d = nc.dram_tensor("iota", [128, 128], F32, kind="ExternalInput")
    out_d = nc.dram_tensor("out", [B_PER_CORE, L, H], F32, kind="ExternalOutput")

    eq = mybir.AluOpType.is_equal
    mul = mybir.AluOpType.mult

    with tile.TileContext(nc) as tc:
        with (
            tc.tile_pool(name="const", bufs=1) as constp,
            tc.tile_pool(name="auxp", bufs=2) as auxp,
            tc.tile_pool(name="hid", bufs=3) as hidp,
            tc.tile_pool(name="band", bufs=4) as bandp,
            tc.tile_pool(name="outp", bufs=2) as outp,
            tc.tile_pool(name="psum", bufs=6, space="PSUM") as psump,
        ):
            iota_t = constp.tile([128, 128], F32)
            nc.sync.dma_start(iota_t[:], iota_d[:])

            for i in range(B_PER_CORE):
                auxt = auxp.tile([128, 64], F32)
                nc.sync.dma_start(auxt[:], aux_d[i])
                ps = []
                for g in range(NGROUP):
                    # 1MB load: 512 timesteps -> [128 part, 4 chunk, 512 h]
                    ht4 = hidp.tile([128, GROUP, H], F32)
                    src = hid_d[i, g * GROUP * TCHUNK : (g + 1) * GROUP * TCHUNK, :]
                    nc.sync.dma_start(ht4[:], src.rearrange("(c p) h -> p c h", p=128))
                    psg = psump.tile([128, H], F32)
                    ps.append(psg)
                    for cc in range(GROUP):
                        c = g * GROUP + cc
                        t1 = bandp.tile([128, 128], F32, tag="t1")
                        nc.vector.tensor_scalar(
                            t1[:], iota_t[:], auxt[:, c : c + 1],
                            auxt[:, 32 + c : 33 + c], eq, mul,
                        )
                        t2 = bandp.tile([128, 128], F32, tag="t2")
                        nc.vector.tensor_scalar(
                            t2[:], iota_t[:], auxt[:, 16 + c : 17 + c],
                            auxt[:, 48 + c : 49 + c], eq, mul,
                        )
                        wt = bandp.tile([128, 128], F32, tag="wt")
                        nc.gpsimd.tensor_add(wt[:], t1[:], t2[:])
                        nc.tensor.matmul(
                            psg[:], wt[:], ht4[:, cc, :],
                            start=(cc == 0), stop=(cc == GROUP - 1),
                        )
                # Combine overlapping windows (slot coverage: g0 0..127,
                # g1 32..159, g2 96..223, g3 128..255). PSUM row = slot
                # mod 128, so every combine slice is partition-aligned.
                acc0 = outp.tile([128, H], F32, tag="acc0")
                nc.scalar.copy(acc0[:], ps[0][:])
                # partition ranges must respect HW alignment (32->max 32,
                # 64->max 64), so the 32:128 add is split in two
                nc.vector.tensor_add(acc0[32:64, :], acc0[32:64, :], ps[1][32:64, :])
                nc.vector.tensor_add(acc0[64:128, :], acc0[64:128, :], ps[1][64:128, :])
                nc.vector.tensor_add(acc0[96:128, :], acc0[96:128, :], ps[2][96:128, :])
                nc.sync.dma_start(out_d[i, 0:128, :], acc0[:])
                acc1 = outp.tile([128, H], F32, tag="acc1")
                nc.scalar.copy(acc1[:], ps[3][:])
                nc.vector.tensor_add(acc1[0:96, :], acc1[0:96, :], ps[2][0:96, :])
                nc.vector.tensor_add(acc1[0:32, :], acc1[0:32, :], ps[1][0:32, :])
                nc.sync.dma_start(out_d[i, 128:256, :], acc1[:])

    nc.compile()
    return nc, out_d.name


def build_program_generic():
    """Fallback: full-width band, two matmuls per chunk. Input-shape only."""
    nc = bacc.Bacc("TRN2", target_bir_lowering=False, debug=False)

    hid_d = nc.dram_tensor("hidden", [B_PER_CORE, T, H], F32, kind="ExternalInput")
    aux_d = nc.dram_tensor("aux", [B_PER_CORE, 128, 64], F32, kind="ExternalInput")
    iota_d = nc.dram_tensor("iota", [128, L], F32, kind="ExternalInput")
    out_d = nc.dram_tensor("out", [B_PER_CORE, L, H], F32, kind="ExternalOutput")

    eq = mybir.AluOpType.is_equal
    mul = mybir.AluOpType.mult

    with tile.TileContext(nc) as tc:
        with (
            tc.tile_pool(name="const", bufs=1) as constp,
            tc.tile_pool(name="auxp", bufs=2) as auxp,
            tc.tile_pool(name="hid", bufs=4) as hidp,
            tc.tile_pool(name="band", bufs=3) as bandp,
            tc.tile_pool(name="outp", bufs=2) as outp,
            tc.tile_pool(name="psum", bufs=2, space="PSUM") as psump,
        ):
            iota_t = constp.tile([128, L], F32)
            nc.sync.dma_start(iota_t[:], iota_d[:])

            for i in range(B_PER_CORE):
                auxt = auxp.tile([128, 64], F32)
                nc.sync.dma_start(auxt[:], aux_d[i])
                ps0 = psump.tile([128, H], F32)
                ps1 = psump.tile([128, H], F32)
                for c in range(NCHUNK):
                    ht = hidp.tile([128, H], F32)
                    nc.sync.dma_start(ht[:], hid_d[i, c * TCHUNK : (c + 1) * TCHUNK, :])
                    t1 = bandp.tile([128, L], F32, tag="t1")
                    nc.vector.tensor_scalar(
                        t1[:], iota_t[:], auxt[:, c : c + 1],
                        auxt[:, 32 + c : 33 + c], eq, mul,
                    )
                    t2 = bandp.tile([128, L], F32, tag="t2")
                    nc.vector.tensor_scalar(
                        t2[:], iota_t[:], auxt[:, 16 + c : 17 + c],
                        auxt[:, 48 + c : 49 + c], eq, mul,
                    )
                    wt = bandp.tile([128, L], F32, tag="wt")
                    nc.gpsimd.tensor_add(wt[:], t1[:], t2[:])
                    nc.tensor.matmul(
                        ps0[:], wt[:, 0:128], ht[:],
                        start=(c == 0), stop=(c == NCHUNK - 1),
                    )
                    nc.tensor.matmul(
                        ps1[:], wt[:, 128:256], ht[:],
                        start=(c == 0), stop=(c == NCHUNK - 1),
                    )
                o0 = outp.tile([128, H], F32, tag="o0")
                nc.scalar.copy(o0[:], ps0[:])
                o1 = outp.tile([128, H], F32, tag="o1")
                nc.scalar.copy(o1[:], ps1[:])
                nc.sync.dma_start(out_d[i, 0:128, :], o0[:])
                nc.sync.dma_start(out_d[i, 128:256, :], o1[:])

    nc.compile()
    return nc, out_d.name


def _get_compiled(variant: str):
    if variant not in _compiled:
        _compiled[variant] = (
            build_program_windowed() if variant == "windowed"
            else build_program_generic()
        )
    return _compiled[variant]


def prepare(hidden: np.ndarray, alphas: np.ndarray):
    """Host scan + input packing. Returns (variant, in_maps)."""
    slotA, slotB, wA, wB = host_scan(alphas)
    if _window_ok(slotA, slotB, wA, wB):
        variant = "windowed"
        # PSUM row = slot mod 128 (window width 128 makes this a bijection)
        aux = _pack_aux(np.mod(slotA, 128), np.mod(slotB, 128), wA, wB)
        iota_cols = 128
    else:
        variant = "generic"
        aux = _pack_aux(slotA, slotB, wA, wB)
        iota_cols = L
    iota = np.ascontiguousarray(
        np.broadcast_to(np.arange(iota_cols, dtype=np.float32), (128, iota_cols))
    )
    in_maps = []
    for j in range(N_CORES):
        sl = slice(j * B_PER_CORE, (j + 1) * B_PER_CORE)
        in_maps.append(
            {
                "hidden": np.ascontiguousarray(hidden[sl]),
                "aux": np.ascontiguousarray(aux[sl]),
                "iota": iota,
            }
        )
    return variant, in_maps


def run_sharded(hidden: np.ndarray, alphas: np.ndarray, trace: bool = False, **kw):
    """Run the SPMD kernel; returns (out [B,L,H] f32, BassKernelResults)."""
    variant, in_maps = prepare(hidden, alphas)
    nc, out_name = _get_compiled(variant)
    res = run_bass_kernel_spmd(nc, in_maps, list(range(N_CORES)), trace=trace, **kw)
    out = np.concatenate([r[out_name] for r in res.results], axis=0)
    return out, res


def kernel(hidden, alphas, num_labels=L) -> np.ndarray:
    hidden = np.asarray(hidden, dtype=np.float32)
    alphas = np.asarray(alphas, dtype=np.float32)
    assert hidden.shape == (B, T, H) and alphas.shape == (B, T)
    assert int(num_labels) == L
    out, _ = run_sharded(hidden, alphas)
    return out
